# revision 11
# baseline (speedup 1.0000x reference)
"""Trainium2 Bass kernel for nn_DSA_32255204393142 (DeepSeek-style sparse attention).

Strategy (8 NeuronCores, fully data-parallel, no collectives):
  core c -> batch b = c//4, query shard rows [s0, s0+512), s0 = (c%4)*512.

Per core:
  1. fp32 indexer projections q_idx = LN(x @ Wq + bq), k_idx = LN(x @ Wk + bk)
     computed feature-major (features on partitions) from host-pre-transposed x.
     LN stats via PE block-diag ones matmuls (sums come out pre-replicated across
     the 64 feature partitions of each head); rstd = exp(-0.5*ln(var+eps)) with
     one Newton refinement step to fp32 accuracy. indexer head weights (0.1) are
     folded into the k-side LN gamma (requires w >= 0, asserted host-side).
  2. M[i, j] = sum_h w_h * relu(k_idx[i,h] . q_idx[j,h])  (512 x 2048, fp32)
  3. Exact per-row top-512 threshold per query-tile: 14 bisection iterations with
     fused compare-and-count (DVE tensor_scalar+accum on a bf16 copy for two
     qtiles, ScalarE Sign+accum for the other two, so the chains overlap), then
     an exact fixup that ranks the boundary candidates by fp32 value with two
     max8 rounds and includes exactly 512 - count(M > hi) of them.
  4. Dense masked attention in fp16 (scores transposed: keys on partitions):
     S^T = K^T-chunk^T @ Q^T, W = exp(S/8) * mask, O'^T = [V | 1]^T-chunks @ W
     accumulated over key chunks. Row 64 of O' is the softmax denominator.
     Host divides and reassembles. No max-subtraction needed (|s/8| <~ 6).

kernel(**inputs) takes the FULL unsharded inputs and returns
(attn_output (2,2048,1024) f32, kl_loss () f32) matching reference.py.
"""

import sys

sys.path.insert(0, "/opt/trn_rl_repo")

import numpy as np
import ml_dtypes  # noqa: F401

import concourse.bass as bass  # noqa: F401
import concourse.bacc as bacc
import concourse.mybir as mybir
from concourse import tile
from concourse import bass_utils

F32 = mybir.dt.float32
BF16 = mybir.dt.bfloat16
FP16 = mybir.dt.float16
U32 = mybir.dt.uint32
OP = mybir.AluOpType
AF = mybir.ActivationFunctionType
AX = mybir.AxisListType

B, S, D = 2, 2048, 1024
H, DK = 16, 64
HI, DI = 4, 64
TOP_K = 512
N_CORES = 8
SHARD = 512
EPS = 1e-5
N_ITERS = 14
BIG = 3.0e38


def _dram(nc, name, shape, dtype, out=False):
    kind = "ExternalOutput" if out else "ExternalInput"
    return nc.dram_tensor(name, list(shape), dtype, kind=kind).ap()


def build_program(debug=False):
    nc = bacc.Bacc("TRN2", target_bir_lowering=False, debug=False,
                   enable_asserts=False, num_devices=N_CORES)

    # ---------------- DRAM I/O ----------------
    xT = _dram(nc, "xT", (D, S), F32)
    xTs = _dram(nc, "xTs", (D, SHARD), F32)
    qT2 = _dram(nc, "qT2", (H // 2, DK, 2 * SHARD), FP16)
    kTb = _dram(nc, "kT", (DK, S), FP16)
    v1t = _dram(nc, "v1t", (S, DK + 1), FP16)
    wq = _dram(nc, "wq", (D, HI * DI), F32)
    wk = _dram(nc, "wk", (D, HI * DI), F32)
    bq = _dram(nc, "bq", (128, 2), F32)
    bk = _dram(nc, "bk", (128, 2), F32)
    gq = _dram(nc, "gq", (128, 2), F32)
    bq_ln = _dram(nc, "bq_ln", (128, 2), F32)
    gk = _dram(nc, "gk", (128, 2), F32)
    bk_ln = _dram(nc, "bk_ln", (128, 2), F32)
    gblk = _dram(nc, "gblk", (128, 128), F32)
    ident = _dram(nc, "ident", (128, 128), FP16)
    o_out = _dram(nc, "o", (H, DK + 1, SHARD), F32, out=True)
    dbg = {}
    if debug:
        dbg["m"] = _dram(nc, "dbg_m", (SHARD, S), F32, out=True)
        dbg["mask"] = _dram(nc, "dbg_mask", (SHARD, S), FP16, out=True)
        dbg["qn"] = _dram(nc, "dbg_qn", (HI * DI, S), F32, out=True)
        dbg["kn"] = _dram(nc, "dbg_kn", (HI * DI, SHARD), F32, out=True)

    with tile.TileContext(nc) as tc:
        persist = tc.alloc_tile_pool(name="persist", bufs=1)
        gq_sb = persist.tile([128, 2], F32, tag="gq", name="gq_sb")
        bqln_sb = persist.tile([128, 2], F32, tag="bqln", name="bqln_sb")
        gk_sb = persist.tile([128, 2], F32, tag="gk", name="gk_sb")
        bkln_sb = persist.tile([128, 2], F32, tag="bkln", name="bkln_sb")
        gblk_sb = persist.tile([128, 128], F32, tag="gblk", name="gblk_sb")
        ident_sb = persist.tile([128, 128], FP16, tag="ident", name="ident_sb")
        nc.sync.dma_start(gq_sb[:], gq[:])
        nc.sync.dma_start(bqln_sb[:], bq_ln[:])
        nc.sync.dma_start(gk_sb[:], gk[:])
        nc.sync.dma_start(bkln_sb[:], bk_ln[:])
        nc.sync.dma_start(gblk_sb[:], gblk[:])
        nc.sync.dma_start(ident_sb[:], ident[:])

        # long-lived pools first (stack allocator: LIFO release)
        maskT_pool = tc.alloc_tile_pool(name="maskT_pool", bufs=1)
        maskT = [maskT_pool.tile([128, SHARD], FP16, tag=f"mt{kt}", name=f"mt{kt}")
                 for kt in range(16)]
        qn_pool = tc.alloc_tile_pool(name="qn_pool", bufs=1)
        qn = [qn_pool.tile([128, S], F32, tag=f"qn{m}", name=f"qn{m}")
              for m in range(2)]
        kn = [qn_pool.tile([128, SHARD], F32, tag=f"kn{m}", name=f"kn{m}")
              for m in range(2)]

        # ---------------- Phase 1: projections ----------------
        qraw_pool = tc.alloc_tile_pool(name="qraw", bufs=1)
        qraw = [qraw_pool.tile([128, S], F32, tag=f"qraw{m}", name=f"qraw{m}")
                for m in range(2)]
        kraw = [qraw_pool.tile([128, SHARD], F32, tag=f"kraw{m}", name=f"kraw{m}")
                for m in range(2)]

        ph1 = tc.alloc_tile_pool(name="ph1", bufs=1)
        ps1 = tc.alloc_tile_pool(name="ps1", bufs=4, space="PSUM")
        xt_sb = [ph1.tile([128, S], F32, tag=f"xt{k}", name=f"xt{k}")
                 for k in range(8)]
        xts_sb = [ph1.tile([128, SHARD], F32, tag=f"xts{k}", name=f"xts{k}")
                  for k in range(8)]
        wq_sb = [ph1.tile([128, HI * DI], F32, tag=f"wq{k}", name=f"wq{k}")
                 for k in range(8)]
        wk_sb = [ph1.tile([128, HI * DI], F32, tag=f"wk{k}", name=f"wk{k}")
                 for k in range(8)]
        bq_sb = ph1.tile([128, 2], F32, tag="bq", name="bq_sb")
        bk_sb = ph1.tile([128, 2], F32, tag="bk", name="bk_sb")
        nc.sync.dma_start(bq_sb[:], bq[:])
        nc.sync.dma_start(bk_sb[:], bk[:])
        for k in range(8):
            nc.sync.dma_start(xt_sb[k][:], xT[k * 128:(k + 1) * 128, :])
            nc.sync.dma_start(xts_sb[k][:], xTs[k * 128:(k + 1) * 128, :])
            nc.sync.dma_start(wq_sb[k][:], wq[k * 128:(k + 1) * 128, :])
            nc.sync.dma_start(wk_sb[k][:], wk[k * 128:(k + 1) * 128, :])

        for m in range(2):
            for n in range(4):
                pq = ps1.tile([128, 512], F32, tag="proj", name="pq")
                for k in range(8):
                    nc.tensor.matmul(pq[:], wq_sb[k][:, m * 128:(m + 1) * 128],
                                     xt_sb[k][:, n * 512:(n + 1) * 512],
                                     start=(k == 0), stop=(k == 7))
                nc.scalar.activation(qraw[m][:, n * 512:(n + 1) * 512], pq[:],
                                     AF.Identity, bias=bq_sb[:, m:m + 1])
            pk = ps1.tile([128, 512], F32, tag="proj", name="pk")
            for k in range(8):
                nc.tensor.matmul(pk[:], wk_sb[k][:, m * 128:(m + 1) * 128],
                                 xts_sb[k][:], start=(k == 0), stop=(k == 7))
            nc.scalar.activation(kraw[m][:], pk[:], AF.Identity,
                                 bias=bk_sb[:, m:m + 1])
        ph1.release()
        ps1.release()

        # ---------------- Phase 2: layernorm ----------------
        scr = tc.alloc_tile_pool(name="ln_scr", bufs=1)
        ps2 = tc.alloc_tile_pool(name="ps2", bufs=4, space="PSUM")
        eps_sb = scr.tile([128, 1], F32, tag="eps", name="eps_sb")
        nc.vector.memset(eps_sb[:], EPS)
        for side, raw, normed, width, g_sb, b_sb in (
                ("q", qraw, qn, S, gq_sb, bqln_sb),
                ("k", kraw, kn, SHARD, gk_sb, bkln_sb)):
            nch = width // 512
            for m in range(2):
                x_ = raw[m]
                sq = scr.tile([128, width], F32, tag="lnA", name="sq")
                nc.scalar.activation(sq[:], x_[:], AF.Square)
                ssum = scr.tile([128, width], F32, tag="lnB", name="ssum")
                sqsum = scr.tile([128, width], F32, tag="lnC", name="sqsum")
                for n in range(nch):
                    p1 = ps2.tile([128, 512], F32, tag="st", name="p1")
                    nc.tensor.matmul(p1[:], gblk_sb[:],
                                     x_[:, n * 512:(n + 1) * 512],
                                     start=True, stop=True)
                    nc.scalar.copy(ssum[:, n * 512:(n + 1) * 512], p1[:])
                    p2 = ps2.tile([128, 512], F32, tag="st", name="p2")
                    nc.tensor.matmul(p2[:], gblk_sb[:],
                                     sq[:, n * 512:(n + 1) * 512],
                                     start=True, stop=True)
                    nc.scalar.copy(sqsum[:, n * 512:(n + 1) * 512], p2[:])
                mean = scr.tile([128, width], F32, tag="lnD", name="mean")
                nc.scalar.mul(mean[:], ssum[:], 1.0 / DI)
                p_ = scr.tile([128, width], F32, tag="lnE", name="p_")
                nc.scalar.activation(p_[:], sqsum[:], AF.Identity,
                                     bias=eps_sb[:], scale=1.0 / DI)
                t_ = scr.tile([128, width], F32, tag="lnF", name="t_")
                nc.scalar.activation(t_[:], mean[:], AF.Square)
                s_ = scr.tile([128, width], F32, tag="lnA", name="s_")
                nc.vector.scalar_tensor_tensor(s_[:], t_[:], -1.0, p_[:],
                                               OP.mult, OP.add)
                ln_ = scr.tile([128, width], F32, tag="lnB", name="ln_")
                nc.scalar.activation(ln_[:], s_[:], AF.Ln)
                r0 = scr.tile([128, width], F32, tag="lnC2", name="r0")
                nc.scalar.activation(r0[:], ln_[:], AF.Exp, scale=-0.5)
                r2 = scr.tile([128, width], F32, tag="lnE", name="r2")
                nc.scalar.activation(r2[:], r0[:], AF.Square)
                u_ = scr.tile([128, width], F32, tag="lnF", name="u_")
                nc.vector.tensor_tensor(out=u_[:], in0=r2[:], in1=s_[:],
                                        op=OP.mult)
                v_ = scr.tile([128, width], F32, tag="lnE", name="v_")
                nc.vector.tensor_scalar(out=v_[:], in0=u_[:], scalar1=-0.5,
                                        scalar2=1.5, op0=OP.mult, op1=OP.add)
                r_ = scr.tile([128, width], F32, tag="lnF", name="r_")
                nc.vector.tensor_tensor(out=r_[:], in0=r0[:], in1=v_[:],
                                        op=OP.mult)
                c_ = scr.tile([128, width], F32, tag="lnB", name="c_")
                nc.vector.tensor_tensor(out=c_[:], in0=mean[:], in1=r_[:],
                                        op=OP.mult)
                t1 = scr.tile([128, width], F32, tag="lnE", name="t1")
                nc.vector.tensor_tensor(out=t1[:], in0=x_[:], in1=r_[:],
                                        op=OP.mult)
                t2 = scr.tile([128, width], F32, tag="lnA", name="t2")
                nc.vector.tensor_tensor(out=t2[:], in0=t1[:], in1=c_[:],
                                        op=OP.subtract)
                nc.scalar.activation(normed[m][:], t2[:], AF.Identity,
                                     bias=b_sb[:, m:m + 1],
                                     scale=g_sb[:, m:m + 1])
        ps2.release()
        scr.release()
        qraw_pool.release()

        if debug:
            for m in range(2):
                nc.sync.dma_start(dbg["qn"][m * 128:(m + 1) * 128, :], qn[m][:])
                nc.sync.dma_start(dbg["kn"][m * 128:(m + 1) * 128, :], kn[m][:])

        # -------- Phase 3+4: M, selection, mask, transpose (per qtile) --------
        mask_pool = tc.alloc_tile_pool(name="mask_pool", bufs=1)
        m_pool = tc.alloc_tile_pool(name="m_pool", bufs=1)
        sel = tc.alloc_tile_pool(name="sel", bufs=2)
        ps3 = tc.alloc_tile_pool(name="ps3", bufs=2, space="PSUM")
        ps4 = tc.alloc_tile_pool(name="ps4", bufs=2, space="PSUM")

        bigt = sel.tile([128, 1], F32, tag="bigt", bufs=1, name="bigt")
        nc.vector.memset(bigt[:], BIG)
        junkb = sel.tile([128, S], BF16, tag="junkb", bufs=1, name="junkb")
        junka = sel.tile([128, S], BF16, tag="junka", bufs=1, name="junka")

        for qt in range(4):
            # ---- M matmuls + relu-accumulate over 4 heads (two 1024-halves)
            m_t = m_pool.tile([128, S], F32, tag=f"m{qt}", name=f"m{qt}")
            for half in range(2):
                hs = slice(half * 1024, (half + 1) * 1024)
                for h in range(4):
                    mch, hq = h // 2, h % 2
                    pm = ps3.tile([128, 1024], F32, tag="pm", name="pm")
                    lhsT = kn[mch][hq * 64:(hq + 1) * 64,
                                   qt * 128:(qt + 1) * 128]
                    for n in range(2):
                        nc.tensor.matmul(
                            pm[:, n * 512:(n + 1) * 512], lhsT,
                            qn[mch][hq * 64:(hq + 1) * 64,
                                    half * 1024 + n * 512:
                                    half * 1024 + (n + 1) * 512],
                            start=True, stop=True)
                    if h == 0:
                        nc.scalar.activation(m_t[:, hs], pm[:], AF.Relu)
                    else:
                        nc.vector.scalar_tensor_tensor(
                            m_t[:, hs], pm[:], 0.0, m_t[:, hs], OP.max, OP.add)
            if debug:
                nc.sync.dma_start(dbg["m"][qt * 128:(qt + 1) * 128, :], m_t[:])

            # ---- bisection state for this qtile
            use_dve = (qt % 2 == 0)
            mb = sel.tile([128, S], BF16, tag=f"mb{qt % 2}", bufs=1,
                          name=f"mb{qt}")
            if use_dve:
                nc.vector.tensor_copy(mb[:], m_t[:])
            hi_t = sel.tile([128, 1], F32, tag=f"hi{qt}", bufs=N_ITERS + 1,
                            name=f"hi{qt}")
            lo_t = sel.tile([128, 1], F32, tag=f"lo{qt}", bufs=N_ITERS + 1,
                            name=f"lo{qt}")
            nc.vector.tensor_reduce(hi_t[:], m_t[:], AX.X, OP.max)
            nc.vector.memset(lo_t[:], 0.0)

            for it in range(N_ITERS):
                s_ = sel.tile([128, 1], F32, tag=f"bs{qt}", name="s_")
                nc.vector.tensor_tensor(out=s_[:], in0=lo_t[:], in1=hi_t[:],
                                        op=OP.add)
                mid = sel.tile([128, 1], F32, tag=f"bm{qt}", name="mid")
                nc.vector.tensor_scalar(out=mid[:], in0=s_[:], scalar1=0.5,
                                        scalar2=None, op0=OP.mult)
                selm = sel.tile([128, 1], U32, tag=f"bsel{qt}", name="selm")
                if use_dve:
                    cnt_i = sel.tile([128, 1], F32, tag=f"bc{qt}", name="cnt_i")
                    nc.vector.tensor_scalar(out=junkb[:], in0=mb[:],
                                            scalar1=mid[:], scalar2=0.0,
                                            op0=OP.is_gt, op1=OP.add,
                                            accum_out=cnt_i[:])
                    nc.vector.tensor_scalar(out=selm[:], in0=cnt_i[:],
                                            scalar1=511.5, scalar2=None,
                                            op0=OP.is_ge)
                else:
                    negmid = sel.tile([128, 1], F32, tag=f"bn{qt}", name="negmid")
                    nc.vector.tensor_scalar(out=negmid[:], in0=s_[:],
                                            scalar1=-0.5, scalar2=None,
                                            op0=OP.mult)
                    acc = sel.tile([128, 1], F32, tag=f"bc{qt}", name="acc")
                    nc.scalar.activation(junka[:], m_t[:], AF.Sign,
                                         bias=negmid[:],
                                         accum_out=acc[:])
                    nc.vector.tensor_scalar(out=selm[:], in0=acc[:],
                                            scalar1=-1024.0, scalar2=None,
                                            op0=OP.is_ge)
                lo_new = sel.tile([128, 1], F32, tag=f"lo{qt}",
                                  bufs=N_ITERS + 1, name="lo_new")
                hi_new = sel.tile([128, 1], F32, tag=f"hi{qt}",
                                  bufs=N_ITERS + 1, name="hi_new")
                nc.vector.select(out=lo_new[:], mask=selm[:], on_true=mid[:],
                                 on_false=lo_t[:])
                nc.vector.select(out=hi_new[:], mask=selm[:], on_true=hi_t[:],
                                 on_false=mid[:])
                lo_t, hi_t = lo_new, hi_new

            # ---- exact fixup + final mask
            mask_t = mask_pool.tile([128, S], FP16, tag=f"mask{qt}",
                                    name=f"mask{qt}")
            hq = hi_t[:]
            # the in/out split must use the same values the bisection counted
            # (bf16 copy for DVE qtiles, fp32 M for ACT qtiles) — bf16 rounding
            # is monotone, so {Mb > hi} is an upper set in fp32 M-order and the
            # fp32-ranked fixup over {Mb <= hi} stays exact.
            m_sel = mb if use_dve else m_t
            gt_hi = sel.tile([128, S], FP16, tag="gt_hi", name="gt_hi")
            nc.vector.tensor_scalar(out=gt_hi[:], in0=m_sel[:], scalar1=hq,
                                    scalar2=None, op0=OP.is_gt)
            cnt = sel.tile([128, 1], F32, tag="cnt", name="cnt")
            nc.vector.tensor_reduce(cnt[:], gt_hi[:], AX.X, OP.add)
            d_ = sel.tile([128, 1], F32, tag="d_", name="d_")
            nc.vector.tensor_scalar(out=d_[:], in0=cnt[:], scalar1=-1.0,
                                    scalar2=float(TOP_K), op0=OP.mult,
                                    op1=OP.add)
            d_ = _clamp16(nc, sel, d_)
            cand = sel.tile([128, S], F32, tag="cand", name="cand")
            nc.vector.scalar_tensor_tensor(cand[:], m_sel[:], hq, m_t[:],
                                           OP.is_le, OP.mult)
            m16 = sel.tile([128, 16], F32, tag="m16", name="m16")
            nc.vector.max(out=m16[:, 0:8], in_=cand[:])
            candz = sel.tile([128, S], F32, tag="candz", name="candz")
            nc.vector.match_replace(out=candz[:], in_to_replace=m16[:, 0:8],
                                    in_values=cand[:], imm_value=0.0)
            nc.vector.max(out=m16[:, 8:16], in_=candz[:])
            oh = sel.tile([128, 16], F32, tag="oh", name="oh")
            for cpos in range(16):
                nc.vector.tensor_scalar(out=oh[:, cpos:cpos + 1], in0=d_[:],
                                        scalar1=float(cpos + 1), scalar2=None,
                                        op0=OP.is_equal)
            thr0 = sel.tile([128, 16], F32, tag="thr0", name="thr0")
            nc.vector.tensor_tensor(out=thr0[:], in0=m16[:], in1=oh[:],
                                    op=OP.mult)
            thr = sel.tile([128, 1], F32, tag="thr", name="thr")
            nc.vector.tensor_reduce(thr[:], thr0[:], AX.X, OP.add)
            dz = sel.tile([128, 1], U32, tag="dz", name="dz")
            nc.vector.tensor_scalar(out=dz[:], in0=d_[:], scalar1=0.5,
                                    scalar2=None, op0=OP.is_le)
            thr2 = sel.tile([128, 1], F32, tag="thr2", name="thr2")
            nc.vector.select(out=thr2[:], mask=dz[:], on_true=bigt[:],
                             on_false=thr[:])
            inc = sel.tile([128, S], FP16, tag="inc", name="inc")
            nc.vector.tensor_scalar(out=inc[:], in0=cand[:], scalar1=thr2[:],
                                    scalar2=None, op0=OP.is_ge)
            nc.vector.tensor_tensor(out=mask_t[:], in0=gt_hi[:], in1=inc[:],
                                    op=OP.add)
            if debug:
                nc.sync.dma_start(dbg["mask"][qt * 128:(qt + 1) * 128, :],
                                  mask_t[:])

            # ---- transpose this qtile's mask into maskT columns
            for kt in range(16):
                pt = ps4.tile([128, 128], FP16, tag="tr", name="pt")
                nc.tensor.transpose(pt[:], mask_t[:, kt * 128:(kt + 1) * 128],
                                    ident_sb[:])
                nc.scalar.copy(maskT[kt][:, qt * 128:(qt + 1) * 128], pt[:])

        ps4.release()
        ps3.release()
        sel.release()
        m_pool.release()
        mask_pool.release()
        qn_pool.release()

        # ---------------- Phase 5: attention ----------------
        attn = tc.alloc_tile_pool(name="attn", bufs=1)
        ps5s = tc.alloc_tile_pool(name="ps5s", bufs=2, space="PSUM")
        ps5a = tc.alloc_tile_pool(name="ps5a", bufs=4, space="PSUM")
        qt2_sb = [attn.tile([DK, 2 * SHARD], FP16, tag=f"qt2_{p}",
                            name=f"qt2_{p}") for p in range(H // 2)]
        kt_sb = attn.tile([DK, S], FP16, tag="kt", name="kt_sb")
        v1_sb = [attn.tile([128, DK + 1], FP16, tag=f"v1_{k}", name=f"v1_{k}")
                 for k in range(16)]
        for p in range(H // 2):
            nc.sync.dma_start(qt2_sb[p][:], qT2[p, :, :])
        nc.sync.dma_start(kt_sb[:], kTb[:])
        for k in range(16):
            nc.sync.dma_start(v1_sb[k][:], v1t[k * 128:(k + 1) * 128, :])

        for g in range(4):
            av = [ps5a.tile([DK + 1, SHARD], F32, tag="av", name="av")
                  for _ in range(4)]
            wms = []
            for kt in range(16):
                wm = attn.tile([128, 4 * SHARD], FP16, tag=f"wm{kt}", bufs=1,
                               name=f"wm{kt}")
                wms.append(wm)
                for pp in range(2):
                    st = ps5s.tile([128, 2 * SHARD], F32, tag="st", name="st")
                    for nn2 in range(2):
                        nc.tensor.matmul(
                            st[:, nn2 * 512:(nn2 + 1) * 512],
                            kt_sb[:, kt * 128:(kt + 1) * 128],
                            qt2_sb[2 * g + pp][:, nn2 * 512:(nn2 + 1) * 512],
                            start=True, stop=True)
                    nc.scalar.activation(wm[:, pp * 1024:(pp + 1) * 1024],
                                         st[:], AF.Exp, scale=0.125)
            for kt in range(16):
                for i in range(4):
                    nc.vector.tensor_tensor(
                        out=wms[kt][:, i * 512:(i + 1) * 512],
                        in0=wms[kt][:, i * 512:(i + 1) * 512],
                        in1=maskT[kt][:], op=OP.mult)
            for kt in range(16):
                for i in range(4):
                    nc.tensor.matmul(av[i][:], v1_sb[kt][:],
                                     wms[kt][:, i * 512:(i + 1) * 512],
                                     start=(kt == 0), stop=(kt == 15))
            for i in range(4):
                h = 4 * g + i
                o_sb = attn.tile([DK + 1, SHARD], F32, tag="o_sb", bufs=4,
                                 name="o_sb")
                nc.scalar.copy(o_sb[:], av[i][:])
                nc.sync.dma_start(o_out[h, :, :], o_sb[:])
        ps5a.release()
        ps5s.release()
        attn.release()
        maskT_pool.release()
        persist.release()

    nc.compile()
    return nc


def _clamp16(nc, sel, d_):
    d2 = sel.tile([128, 1], F32, tag="d2", name="d2")
    nc.vector.tensor_scalar(out=d2[:], in0=d_[:], scalar1=16.0, scalar2=None,
                            op0=OP.min)
    return d2


_CACHE = {}


def _get_program(debug=False):
    key = bool(debug)
    if key not in _CACHE:
        _CACHE[key] = build_program(debug=debug)
    return _CACHE[key]


def make_in_maps(x, Q, K, V, Wq_idx, bq_idx, Wk_idx, bk_idx, ln_gamma, ln_beta,
                 indexer_weights):
    x = np.asarray(x, np.float32)
    Q = np.asarray(Q, np.float32)
    K = np.asarray(K, np.float32)
    V = np.asarray(V, np.float32)
    Wq_idx = np.asarray(Wq_idx, np.float32)
    bq_idx = np.asarray(bq_idx, np.float32)
    Wk_idx = np.asarray(Wk_idx, np.float32)
    bk_idx = np.asarray(bk_idx, np.float32)
    ln_gamma = np.asarray(ln_gamma, np.float32)
    ln_beta = np.asarray(ln_beta, np.float32)
    w_idx = np.asarray(indexer_weights, np.float32)
    assert np.all(w_idx >= 0), "indexer weight folding requires w >= 0"

    hp = np.float16

    def chunk_col(v):
        return np.ascontiguousarray(v.reshape(2, 128).T)

    gq = chunk_col(np.tile(ln_gamma, HI))
    bq_ln = chunk_col(np.tile(ln_beta, HI))
    gk = chunk_col(np.tile(ln_gamma, HI) * np.repeat(w_idx, DI))
    bk_ln = chunk_col(np.tile(ln_beta, HI) * np.repeat(w_idx, DI))
    bqc = chunk_col(bq_idx)
    bkc = chunk_col(bk_idx)
    gblk = np.zeros((128, 128), np.float32)
    gblk[:64, :64] = 1.0
    gblk[64:, 64:] = 1.0
    ident = np.eye(128, dtype=np.float32).astype(hp)

    in_maps = []
    for c in range(N_CORES):
        b = c // 4
        s0 = (c % 4) * SHARD
        xTb = np.ascontiguousarray(x[b].T)
        xTs = np.ascontiguousarray(xTb[:, s0:s0 + SHARD])
        qT2 = np.empty((H // 2, DK, 2 * SHARD), np.float32)
        for p in range(H // 2):
            for i in range(2):
                h = 2 * p + i
                qT2[p, :, i * SHARD:(i + 1) * SHARD] = Q[b, h, s0:s0 + SHARD, :].T
        v1 = np.concatenate([V[b], np.ones((S, 1), np.float32)], axis=1)
        in_maps.append({
            "xT": xTb, "xTs": xTs,
            "qT2": qT2.astype(hp),
            "kT": np.ascontiguousarray(K[b].T).astype(hp),
            "v1t": v1.astype(hp),
            "wq": Wq_idx, "wk": Wk_idx, "bq": bqc, "bk": bkc,
            "gq": gq, "bq_ln": bq_ln, "gk": gk, "bk_ln": bk_ln,
            "gblk": gblk, "ident": ident,
        })
    return in_maps


def _ensure_ntff_hook():
    """Register the axon NTFF profile hook if the image's antenv lacks it."""
    import types
    try:
        from antenv.axon_hooks import get_axon_ntff_profile_hook  # noqa: F401
        return
    except ImportError:
        pass
    try:
        import antenv
        sys.path.insert(0, "/root/.axon_site")
        from trn_agent_boot.trn_boot import _ntff_profile_via_ctypes
        hook = _ntff_profile_via_ctypes("/opt/axon/libaxon_pjrt.so")
        mod = types.ModuleType("antenv.axon_hooks")
        mod.get_axon_ntff_profile_hook = lambda: hook
        mod.set_axon_ntff_profile_hook = lambda h: None
        sys.modules["antenv.axon_hooks"] = mod
        antenv.axon_hooks = mod
    except Exception:
        pass


def run_cores(in_maps, debug=False, **run_kwargs):
    if run_kwargs.get("trace"):
        _ensure_ntff_hook()
    nc = _get_program(debug=debug)
    return bass_utils.run_bass_kernel_spmd(
        nc, in_maps, core_ids=list(range(N_CORES)), **run_kwargs)


def assemble_output(results):
    out = np.empty((B, S, H * DK), np.float32)
    for c in range(N_CORES):
        b = c // 4
        s0 = (c % 4) * SHARD
        o = results[c]["o"]
        num = o[:, :DK, :]
        z = o[:, DK, :]
        attn = (num / z[:, None, :]).transpose(2, 0, 1).reshape(SHARD, H * DK)
        out[b, s0:s0 + SHARD, :] = attn
    return out


def kernel(**inputs):
    in_maps = make_in_maps(**inputs)
    res = run_cores(in_maps)
    out = assemble_output(res.results)
    return out, np.zeros((), np.float32)


if __name__ == "__main__":
    rng = np.random.default_rng(0)
    ins = {
        "x": rng.standard_normal((B, S, D), dtype=np.float32),
        "Q": rng.standard_normal((B, H, S, DK), dtype=np.float32),
        "K": rng.standard_normal((B, S, DK), dtype=np.float32),
        "V": rng.standard_normal((B, S, DK), dtype=np.float32),
        "Wq_idx": (rng.standard_normal((D, HI * DI), dtype=np.float32) * 0.02),
        "bq_idx": np.zeros(HI * DI, np.float32),
        "Wk_idx": (rng.standard_normal((D, HI * DI), dtype=np.float32) * 0.02),
        "bk_idx": np.zeros(HI * DI, np.float32),
        "ln_gamma": np.ones(DI, np.float32),
        "ln_beta": np.zeros(DI, np.float32),
        "indexer_weights": np.full(HI, 0.1, np.float32),
    }
    out, kl = kernel(**ins)
    print("out", out.shape, out.dtype, "finite:", np.isfinite(out).all())
